# revision 6
# baseline (speedup 1.0000x reference)
"""CLUB mutual-information upper bound (loss_fn) on 8 Trainium2 NeuronCores.

Math: reference computes
    h  = relu(x1 @ W1 + b1); h = relu(h @ W2 + b2); g = tanh(h @ W3 + b3)
    mu, logvar = split(g); iv = exp(-logvar)
    pos = -0.5 (mu - x2)^2 iv
    neg = -0.5 mean_j[(mu_i - x2_j)^2] iv
    mi  = mean_i sum_d (pos - neg)

The O(N^2 D) pairwise term collapses with m1 = mean_j x2, m2 = mean_j x2^2:
    pos - neg = -0.5 iv [x2_i^2 - m2 - 2 mu (x2_i - m1)]
which further decomposes into per-core-local reductions (rows sharded 128/core):
    S0_d = sum_i iv          S1_d = sum_i mu*iv
    T0_d = sum_i iv*x2^2     T1_d = sum_i mu*iv*x2
    p1_d = sum_j x2          p2_d = sum_j x2^2
    N * mi = sum_d [ -0.5*T0 + 0.5*m2*S0 + T1 - m1*S1 ],  m1 = p1/N, m2 = p2/N
so each core needs ONLY its own 128-row shard of x1/x2 plus the (replicated)
weights: data-parallel, no collectives, cross-core coupling resolved on host.

Device layout is feature-major ([feature partitions, row free-axis]); the host
packs pre-transposed shards + weights into one blob so the kernel is a single
input DMA, 12 fp32 matmuls, 7 activations, 5 vector ops, 1 output DMA.
"""

import sys

import numpy as np

sys.path.insert(0, "/opt/trn_rl_repo")

import concourse.bass as bass
import concourse.tile as tile
from concourse import mybir
from concourse.bass_utils import run_bass_kernel_spmd

DT = mybir.dt.float32
NCORES = 8
N = 1024
X1D = 256
X2D = 128
HID = 256
ROWS = N // NCORES  # 128
P = 128

# blob (per-core): [128 partitions, 1926] f32
#   [0:256)     x1sT   col k*128+j   = x1s[j, k*128+p]
#   [256:384)   x2sT   col 256+j     = x2s[j, p]
#   [384:390)   biases col 384+2l+m  = b_l[m*128+p]
#   [390:1926)  W      col 390+l*512+k*256+j = W_l[k*128+p, j]
X2T_OFF = 256
B_OFF = 384
W_OFF = 390
BLOB_W = W_OFF + 3 * 512  # 1926

_module_cache = None


def _build_module():
    nc = bass.Bass()
    blob = nc.declare_dram_parameter("blob", [P, BLOB_W], DT, isOutput=False)
    out = nc.declare_dram_parameter("out", [P, 6], DT, isOutput=True)

    AF = mybir.ActivationFunctionType
    ALU = mybir.AluOpType

    with tile.TileContext(nc) as tc:
        with (
            tc.tile_pool(name="sb", bufs=1) as sb,
            tc.tile_pool(name="ps", bufs=4, space="PSUM") as ps,
        ):
            bsb = sb.tile([P, BLOB_W], DT, tag="blob")
            nc.sync.dma_start(out=bsb[:], in_=blob[:])

            out_sb = sb.tile([P, 6], DT, tag="outsb")

            # This walrus build allows one sync-wait per compute instruction.
            # Touch the blob on ACT first so its engine clock observes the
            # input DMA; later activations then only wait on PE.
            warm = sb.tile([1, 1], DT, tag="warm")
            nc.scalar.copy(out=warm[:], in_=bsb[0:1, 0:1])

            x1T = [bsb[:, k * 128 : (k + 1) * 128] for k in range(2)]
            x2T = bsb[:, X2T_OFF : X2T_OFF + ROWS]

            def w_ap(l, k, m):
                c = W_OFF + l * 512 + k * 256 + m * 128
                return bsb[:, c : c + 128]

            def bias_ap(l, m):
                c = B_OFF + 2 * l + m
                return bsb[:, c : c + 1]

            # x2 shard stats: p1 = col-sums, p2 = col-sums of squares
            # (x2sq kept for T0 below)
            nc.vector.reduce_sum(
                out=out_sb[:, 2:3], in_=x2T, axis=mybir.AxisListType.X
            )
            x2sq = sb.tile([P, ROWS], DT, tag="x2sq")
            nc.vector.scalar_tensor_tensor(
                out=x2sq[:],
                in0=x2T,
                scalar=1.0,
                in1=x2T,
                op0=ALU.bypass,
                op1=ALU.mult,
                accum_out=out_sb[:, 3:4],
            )

            # MLP, feature-major: h_next[m] = act(sum_k W[k,m-slice].T @ h[k] + b[m])
            h = x1T
            for l in range(3):
                nxt = []
                for m in range(2):
                    pt = ps.tile([P, ROWS], DT, tag="mm")
                    for k in range(2):
                        nc.tensor.matmul(
                            pt[:],
                            lhsT=w_ap(l, k, m),
                            rhs=h[k],
                            start=(k == 0),
                            stop=(k == 1),
                        )
                    if l < 2:
                        hm = sb.tile([P, ROWS], DT, tag=f"h{l}{m}")
                        nc.scalar.activation(
                            out=hm[:],
                            in_=pt[:],
                            func=AF.Relu,
                            bias=bias_ap(l, m),
                            scale=1.0,
                        )
                        nxt.append(hm)
                    else:
                        nxt.append(pt)
                h = nxt

            mu = sb.tile([P, ROWS], DT, tag="mu")
            nc.scalar.activation(
                out=mu[:], in_=h[0][:], func=AF.Tanh, bias=bias_ap(2, 0), scale=1.0
            )
            lv = sb.tile([P, ROWS], DT, tag="lv")
            nc.scalar.activation(
                out=lv[:], in_=h[1][:], func=AF.Tanh, bias=bias_ap(2, 1), scale=1.0
            )
            iv = sb.tile([P, ROWS], DT, tag="iv")
            nc.scalar.activation(out=iv[:], in_=lv[:], func=AF.Exp, scale=-1.0)

            # All out_sb columns are written by DVE so the output DMA waits on
            # a single engine. S0 = sum iv:
            nc.vector.reduce_sum(
                out=out_sb[:, 0:1], in_=iv[:], axis=mybir.AxisListType.X
            )

            # wmi = mu*iv (accum S1), T0 = sum iv*x2^2, T1 = sum wmi*x2
            wmi = sb.tile([P, ROWS], DT, tag="wmi")
            nc.vector.scalar_tensor_tensor(
                out=wmi[:],
                in0=mu[:],
                scalar=1.0,
                in1=iv[:],
                op0=ALU.bypass,
                op1=ALU.mult,
                accum_out=out_sb[:, 1:2],
            )
            scr0 = sb.tile([P, ROWS], DT, tag="scr0")
            nc.vector.scalar_tensor_tensor(
                out=scr0[:],
                in0=iv[:],
                scalar=1.0,
                in1=x2sq[:],
                op0=ALU.bypass,
                op1=ALU.mult,
                accum_out=out_sb[:, 4:5],
            )
            scr1 = sb.tile([P, ROWS], DT, tag="scr1")
            nc.vector.scalar_tensor_tensor(
                out=scr1[:],
                in0=wmi[:],
                scalar=1.0,
                in1=x2T,
                op0=ALU.bypass,
                op1=ALU.mult,
                accum_out=out_sb[:, 5:6],
            )

            nc.sync.dma_start(out=out[:], in_=out_sb[:])
    _split_multi_waits(nc)
    return nc


def _split_multi_waits(nc):
    """This walrus build encodes at most one sync-wait per instruction.
    Hoist extra waits onto same-engine NoOps immediately preceding the
    instruction (engines execute their stream in order, so this is
    semantically identical)."""
    for fn in nc.m.functions:
        for bb in fn.blocks:
            new_insts = []
            for ins in bb.instructions:
                si = ins.sync_info
                if si is not None and len(si.on_wait) > 1:
                    waits = list(si.on_wait)
                    for j, w in enumerate(waits[:-1]):
                        nop = mybir.InstNoOp(
                            name=f"{ins.name}-sw{j}",
                            sync_info=mybir.SyncInfo(on_wait=[w], on_update=[]),
                            bass_nofuse=True,
                            engine=ins.engine,
                        )
                        new_insts.append(nop)
                    si.on_wait = [waits[-1]]
                new_insts.append(ins)
            if len(new_insts) != len(bb.instructions):
                bb.instructions[:] = new_insts


def _pack_inputs(x1, x2, W1, b1, W2, b2, W3, b3):
    f32 = np.float32
    wsec = np.empty((P, 3 * 512), f32)
    for l, W in enumerate((W1, W2, W3)):
        W = np.ascontiguousarray(W, f32)
        for k in range(2):
            wsec[:, l * 512 + k * 256 : l * 512 + (k + 1) * 256] = W[
                k * 128 : (k + 1) * 128, :
            ]
    in_maps = []
    for c in range(NCORES):
        blob = np.empty((P, BLOB_W), f32)
        x1s = np.asarray(x1[c * ROWS : (c + 1) * ROWS], f32)
        x2s = np.asarray(x2[c * ROWS : (c + 1) * ROWS], f32)
        blob[:, 0:128] = x1s[:, 0:128].T
        blob[:, 128:256] = x1s[:, 128:256].T
        blob[:, X2T_OFF : X2T_OFF + ROWS] = x2s.T
        for l, b in enumerate((b1, b2, b3)):
            b = np.asarray(b, f32)
            for m in range(2):
                blob[:, B_OFF + 2 * l + m] = b[m * 128 : (m + 1) * 128]
        blob[:, W_OFF:] = wsec
        in_maps.append({"blob": blob})
    return in_maps


def _run(in_maps, **kwargs):
    global _module_cache
    if _module_cache is None:
        _module_cache = _build_module()
    return run_bass_kernel_spmd(
        _module_cache, in_maps, core_ids=list(range(NCORES)), **kwargs
    )


def _combine(results):
    # cols: 0=S0, 1=S1, 2=p1, 3=p2, 4=T0, 5=T1
    acc = np.zeros((P, 6), np.float64)
    for r in results:
        acc += np.asarray(r["out"], np.float64)
    S0, S1, p1, p2, T0, T1 = (acc[:, i] for i in range(6))
    m1 = p1 / N
    m2 = p2 / N
    total = np.sum(-0.5 * T0 + 0.5 * m2 * S0 + T1 - m1 * S1)
    return np.float32(total / N)


def kernel(x1, x2, W1, b1, W2, b2, W3, b3):
    in_maps = _pack_inputs(x1, x2, W1, b1, W2, b2, W3, b3)
    res = _run(in_maps)
    return _combine(res.results)
